# revision 26
# baseline (speedup 1.0000x reference)
"""Trainium2 Bass kernel for nn_ContextualCritic (4-layer strided conv + segment mean).

Self-contained: kernel(**inputs) -> np.ndarray [2B, 8192].

Design (per core, data-parallel over 8 cores, 512 images each), bf16 matmuls:
 - L1 (3->64, 5x5 s2): host im2col to K=75, zero-padded to K=128; weights
   duplicated along M so the output lands twice in PSUM partitions (0-63 and
   64-127). One N=1024 activation per image (2-bank PSUM tile) amortizes the
   ACT pipeline-fill overhead.
 - L2 (64->128): 25 taps as interleaved K=64 matmul pairs on PE row groups
   (0,0)/(64,0) into two PSUM banks, plus the odd tap as one K=128 matmul
   with zeroed high weight rows; DVE adds banks, two ACTs (one per output
   column parity) write the L2 output in a column-phase, image-inner layout.
 - L3 (128->256), L4 (256->512): inputs stored column-phase + image-innermost
   so every conv tap's moving operand is a contiguous 16B-aligned stream
   (full-rate PE streaming); K=128 accumulation matmuls over taps; w4 stays
   resident in SBUF (loaded once).
 - All matmul operands bf16 (fp32 PSUM accumulate; FWL active), biases fp32,
   final features fp32.
 - Segment mean on host from [N,8192] features (sorted segment ids).
"""
import os
import numpy as np

BLK = 8        # images per Phase-A block
GRP = 32       # images per L4 group (N = GRP*16 = 512)
NCORES = 8

_CACHE = {}


def _build_program(nimg, debug=False):
    from concourse import bacc, mybir
    import concourse.tile as tile

    BF16 = mybir.dt.bfloat16
    F32 = mybir.dt.float32
    LRELU = mybir.ActivationFunctionType.Prelu

    nblk = nimg // BLK
    ngrp = nimg // GRP

    nc = bacc.Bacc(None, target_bir_lowering=False)

    icd = nc.dram_tensor("ic", [96, nimg * 1024], BF16, kind="ExternalInput")
    w1d = nc.dram_tensor("w1", [128, 128], BF16, kind="ExternalInput")
    zd = nc.dram_tensor("zz", [128, 10368], BF16, kind="ExternalInput")
    w2d = nc.dram_tensor("w2", [128, 25 * 128], BF16, kind="ExternalInput")
    w3d = nc.dram_tensor("w3", [128, 2 * 25 * 128], BF16, kind="ExternalInput")
    w4d = nc.dram_tensor("w4", [128, 50 * 512], BF16, kind="ExternalInput")
    b1d = nc.dram_tensor("b1", [128, 1], F32, kind="ExternalInput")
    b2d = nc.dram_tensor("b2", [128, 1], F32, kind="ExternalInput")
    b3d = nc.dram_tensor("b3", [128, 2], F32, kind="ExternalInput")
    b4d = nc.dram_tensor("b4", [128, 4], F32, kind="ExternalInput")
    # f[p, q, grp, (r c i)] -> channel co = q*128+p, feature co*16+r*4+c,
    # image grp*GRP+i
    fd = nc.dram_tensor("f", [128, 4, ngrp, 512], F32, kind="ExternalOutput")

    with tile.TileContext(nc) as tc:
        with tc.tile_pool(name="const", bufs=1) as cst, \
             tc.tile_pool(name="dram", bufs=1, space="DRAM") as drp:
            # per 8-image block gb: [16 r, 2 cpar, 8 c2, 8 img]
            l2od = drp.tile([128, nblk, 2048], BF16)

            w1t = cst.tile([128, 128], BF16)
            nc.gpsimd.dma_start(w1t[:], w1d[:, :])
            w2t = cst.tile([128, 25 * 128], BF16)
            nc.gpsimd.dma_start(w2t[:], w2d[:, :])
            w3t = cst.tile([128, 2 * 25 * 128], BF16)
            nc.gpsimd.dma_start(w3t[:], w3d[:, :])
            w4t = cst.tile([128, 50 * 512], BF16)
            nc.gpsimd.dma_start(w4t[:], w4d[:, :])
            b1t = cst.tile([128, 1], F32)
            nc.gpsimd.dma_start(b1t[:], b1d[:, :])
            b2t = cst.tile([128, 1], F32)
            nc.gpsimd.dma_start(b2t[:], b2d[:, :])
            b3t = cst.tile([128, 2], F32)
            nc.gpsimd.dma_start(b3t[:], b3d[:, :])
            b4t = cst.tile([128, 4], F32)
            nc.gpsimd.dma_start(b4t[:], b4d[:, :])
            a2t = cst.tile([128, 1], F32)
            nc.vector.memset(a2t[:], 0.2)

            # ---------------- Phase A: L1 + L2 ----------------
            # l2i: per image pair [36 r, 2 cpar, 18 c2, 2 img] so L2 taps
            # read 32-element contiguous segments.
            with tc.tile_pool(name="pa", bufs=1) as pa, \
                 tc.tile_pool(name="paps", bufs=2, space="PSUM") as paps, \
                 tc.tile_pool(name="past", bufs=3) as past:
                icT = [pa.tile([128, BLK * 1024], BF16, name=f"ic{i}")
                       for i in range(2)]
                l2iT = [pa.tile([128, BLK, 36, 36], BF16, name=f"l2i{i}")
                        for i in range(2)]

                def pad_init(i):
                    # ic rows 75:96 are zero on the host; no overlap with DMA
                    nc.vector.memset(icT[i][96:128, :], 0.0)
                    nc.vector.memset(
                        l2iT[i][:].rearrange("p i r c -> p (i r c)"), 0.0)

                def l1_img(ic, l2i, img):
                    ps = paps.tile([128, 2, 16, 32], F32, tag="l1ps")
                    for h in range(2):
                        nc.tensor.matmul(
                            ps[:, h, :, :], w1t[:, :],
                            ic[:, (2 * img + h) * 512:(2 * img + h + 1) * 512],
                            start=True, stop=True)
                    nc.scalar.activation(
                        l2i[:, img, 2:34, 2:34],
                        ps[:].rearrange("p h r c -> p (h r) c"),
                        LRELU, bias=b1t[:, :], alpha=a2t[:, :])

                def l2_psb(l2i, ob, psb):
                    j0 = 2 * psb
                    # tap 24 rides the h0 stream on even psbs, h64 on odd —
                    # it overlaps the opposite row-group's stream
                    t24h = psb % 2
                    psA = paps.tile([128, 512], F32, tag="l2psA")
                    psB = paps.tile([128, 512], F32, tag="l2psB")
                    for i in range(12):
                        tA, tB = 2 * i, 2 * i + 1
                        ka, wa = tA // 5, tA % 5
                        kb, wb = tB // 5, tB % 5
                        nc.tensor.matmul(
                            psA[:, :], w2t[0:64, tA * 128:(tA + 1) * 128],
                            l2i[0:64, j0:j0 + 2, ka:ka + 32:2, wa:wa + 32:2],
                            start=(i == 0), stop=(i == 11 and t24h == 1))
                        nc.tensor.matmul(
                            psB[:, :], w2t[64:128, tB * 128:(tB + 1) * 128],
                            l2i[64:128, j0:j0 + 2, kb:kb + 32:2, wb:wb + 32:2],
                            start=(i == 0), stop=(i == 11 and t24h == 0),
                            tile_position=(64, 0))
                    if t24h == 0:
                        nc.tensor.matmul(
                            psA[:, :], w2t[0:64, 24 * 128:25 * 128],
                            l2i[0:64, j0:j0 + 2, 4:36:2, 4:36:2],
                            start=False, stop=True)
                    else:
                        nc.tensor.matmul(
                            psB[:, :], w2t[64:128, 24 * 128:25 * 128],
                            l2i[64:128, j0:j0 + 2, 4:36:2, 4:36:2],
                            start=False, stop=True,
                            tile_position=(64, 0))
                    tb = past.tile([128, 512], F32, tag="l2tb")
                    nc.vector.tensor_copy(tb[:], psB[:, :])
                    st = past.tile([128, 512], F32, tag="l2st")
                    nc.vector.tensor_tensor(st[:], psA[:, :], tb[:],
                                            op=mybir.AluOpType.add)
                    # st flat = (i, r, c2, two); write ob[r, two, c2, j0+i]
                    sv = st[:].rearrange("p (i r c two) -> p r c two i",
                                         i=2, r=16, c=8)
                    for two in range(2):
                        nc.scalar.activation(
                            ob[:, :, two, :, j0:j0 + 2], sv[:, :, :, two, :],
                            LRELU, bias=b2t[:, :], alpha=a2t[:, :])

                def ic_dma(b):
                    ic = icT[b % 2]
                    c0 = b * BLK * 1024
                    nc.sync.dma_start(ic[64:96, :], icd[64:96, c0:c0 + BLK * 1024])
                    nc.sync.dma_start(ic[0:64, :], icd[0:64, c0:c0 + BLK * 1024])

                # software pipeline: L1 of block b+1 hides behind L2 of block b
                pad_init(0)
                pad_init(1)
                ic_dma(0)
                for img in range(BLK):
                    l1_img(icT[0], l2iT[0], img)
                for b in range(nblk):
                    nxt = b + 1 < nblk
                    if nxt:
                        ic_dma(b + 1)
                    ob = past.tile([128, 16, 2, 8, 8], BF16, tag="l2ob")
                    for j in range(4):
                        l2_psb(l2iT[b % 2], ob, j)
                        if nxt:
                            l1_img(icT[(b + 1) % 2], l2iT[(b + 1) % 2], 2 * j)
                            l1_img(icT[(b + 1) % 2], l2iT[(b + 1) % 2],
                                   2 * j + 1)
                    nc.sync.dma_start(
                        l2od[:, b, :],
                        ob[:].rearrange("p r t c i -> p (r t c i)"))

            # ---------------- Phase B: L3 + L4 ----------------
            # l3i: [20 r, 2 cpar, 10 c2, 8 img]; taps 16B-aligned, img-inner
            # l4i: [12 r, 2 cpar, 6 c2, 32 img]
            with tc.tile_pool(name="pb", bufs=1) as pb, \
                 tc.tile_pool(name="pbps", bufs=1, space="PSUM") as pbps, \
                 tc.tile_pool(name="pbst", bufs=3) as pbst:
                l3iT = [pb.tile([128, 20, 2, 10, BLK], BF16, name=f"l3i{i}")
                        for i in range(2)]
                l4iT = [pb.tile([128, 12, 2, 6, GRP], BF16, name=f"l4i{i}")
                        for i in range(2)]
                for i in range(2):
                    nc.vector.memset(
                        l3iT[i][:].rearrange("p r t c i -> p (r t c i)"), 0.0)
                    nc.vector.memset(
                        l4iT[i][:].rearrange("p r t c i -> p (r t c i)"), 0.0)
                l4ps = [pbps.tile([128, 4, 4, GRP], F32, name=f"l4ps{q}")
                        for q in range(4)]

                for grp in range(ngrp):
                    for sb4 in range(4):
                        gb = grp * 4 + sb4
                        l3i = l3iT[gb % 2]
                        lv = l2od[:, gb, :].rearrange(
                            "p (r t ci) -> p r t ci", r=16, t=2)
                        for two in range(2):
                            nc.sync.dma_start(
                                l3i[:, 2:18, two, 1:9, :].rearrange(
                                    "p r c i -> p r (c i)"),
                                lv[:, :, two, :])
                        for cp in range(2):
                            ps3 = pbps.tile([128, 8, 8, BLK], F32,
                                            tag=f"l3ps{cp}")
                            for tap in range(25):
                                kh, kw = tap // 5, tap % 5
                                nc.tensor.matmul(
                                    ps3[:, :, :, :],
                                    w3t[:, (cp * 25 + tap) * 128:
                                        (cp * 25 + tap + 1) * 128],
                                    l3i[:, kh:kh + 16:2, kw % 2,
                                        kw // 2:kw // 2 + 8, :],
                                    start=(tap == 0), stop=(tap == 24))
                            for two in range(2):
                                nc.scalar.activation(
                                    l4iT[cp][:, 2:10, two, 1:5,
                                             sb4 * BLK:(sb4 + 1) * BLK],
                                    ps3[:, :, two::2, :], LRELU,
                                    bias=b3t[:, cp:cp + 1], alpha=a2t[:, :])
                    # L4 over the 32-image group (w4 resident in SBUF)
                    for i4 in range(50):
                        cip, tap = i4 // 25, i4 % 25
                        kh, kw = tap // 5, tap % 5
                        for q in range(4):
                            nc.tensor.matmul(
                                l4ps[q][:, :, :, :],
                                w4t[:, i4 * 512 + q * 128:
                                    i4 * 512 + (q + 1) * 128],
                                l4iT[cip][:, kh:kh + 8:2, kw % 2,
                                          kw // 2:kw // 2 + 4, :],
                                start=(i4 == 0), stop=(i4 == 49))
                    for q in range(4):
                        fo = pbst.tile([128, 512], F32, tag="fo")
                        nc.scalar.activation(
                            fo[:], l4ps[q][:].rearrange("p r c i -> p (r c i)"),
                            LRELU, bias=b4t[:, q:q + 1], alpha=a2t[:, :])
                        nc.sync.dma_start(fd[:, q, grp, :], fo[:])
    nc.compile()
    return nc


def _prep_inputs(x, W1, b1, W2, b2, W3, b3, W4, b4, nimg):
    """Host preprocessing -> per-core in_maps (shared weight arrays)."""
    import ml_dtypes
    f32 = np.float32
    bf16 = ml_dtypes.bfloat16
    n = x.shape[0]
    ncores = n // nimg
    xpad = np.pad(np.asarray(x, dtype=f32), ((0, 0), (0, 0), (2, 2), (2, 2)))
    s = xpad.strides
    v = np.lib.stride_tricks.as_strided(
        xpad, shape=(n, 3, 5, 5, 32, 32),
        strides=(s[0], s[1], s[2], s[3], 2 * s[2], 2 * s[3]))
    # [96, n, 1024] (rows 75:96 zero -- K padding lives on the host)
    ic_all = np.zeros((96, n, 1024), bf16)
    ic_all[0:75] = v.transpose(1, 2, 3, 0, 4, 5).reshape(75, n, 1024)

    w1l = np.ascontiguousarray(
        np.asarray(W1, f32).transpose(1, 2, 3, 0).reshape(75, 64))
    w1h = np.zeros((128, 128), f32)
    w1h[0:75, 0:64] = w1l
    w1h[0:75, 64:128] = w1l
    zz = np.zeros((128, 10368), bf16)
    b1h = np.concatenate([b1, b1]).astype(f32).reshape(128, 1)

    w2h = np.zeros((128, 25 * 128), f32)
    for t in range(25):
        kh, kw = t // 5, t % 5
        lhs = np.asarray(W2, f32)[:, :, kh, kw].T                # [64,128]
        w2h[0:64, t * 128:(t + 1) * 128] = lhs
        w2h[64:128, t * 128:(t + 1) * 128] = lhs
    b2h = np.asarray(b2, f32).reshape(128, 1)

    w3h = np.zeros((128, 2 * 25 * 128), f32)
    for cp in range(2):
        for t in range(25):
            kh, kw = t // 5, t % 5
            w3h[:, (cp * 25 + t) * 128:(cp * 25 + t + 1) * 128] = \
                np.asarray(W3, f32)[cp * 128:(cp + 1) * 128, :, kh, kw].T
    b3h = np.ascontiguousarray(
        np.asarray(b3, f32).reshape(2, 128).T)                   # [128,2]

    w4h = np.zeros((128, 50 * 512), f32)
    for cip in range(2):
        for t in range(25):
            kh, kw = t // 5, t % 5
            i4 = cip * 25 + t
            w4h[:, i4 * 512:(i4 + 1) * 512] = \
                np.asarray(W4, f32)[:, cip * 128:(cip + 1) * 128, kh, kw].T
    b4h = np.ascontiguousarray(
        np.asarray(b4, f32).reshape(4, 128).T)                   # [128,4]

    w1h = w1h.astype(bf16)
    w2h = w2h.astype(bf16)
    w3h = w3h.astype(bf16)
    w4h = w4h.astype(bf16)

    in_maps = []
    for c in range(ncores):
        ic = np.ascontiguousarray(
            ic_all[:, c * nimg:(c + 1) * nimg, :].reshape(96, nimg * 1024))
        in_maps.append({"ic": ic, "w1": w1h, "w2": w2h, "w3": w3h,
                        "w4": w4h, "b1": b1h, "b2": b2h, "b3": b3h,
                        "b4": b4h, "zz": zz})
    return in_maps


def _run(inputs, trace=False, nimg=512, ncores=NCORES):
    from concourse.bass_utils import run_bass_kernel_spmd

    key = (nimg, ncores)
    if key not in _CACHE:
        _CACHE[key] = _build_program(nimg)
    nc = _CACHE[key]

    in_maps = _prep_inputs(
        inputs["x"], inputs["W1"], inputs["b1"], inputs["W2"], inputs["b2"],
        inputs["W3"], inputs["b3"], inputs["W4"], inputs["b4"], nimg)

    res = run_bass_kernel_spmd(nc, in_maps, core_ids=list(range(ncores)),
                               trace=trace)
    ngrp = nimg // GRP
    feats = np.concatenate(
        [r["f"].reshape(128, 4, ngrp, 4, 4, GRP)
         .transpose(2, 5, 1, 0, 3, 4).reshape(nimg, 8192)
         for r in res.results], axis=0)                          # [N, 8192]
    return feats, res


def kernel(**inputs):
    x = np.asarray(inputs["x"])
    n = x.shape[0]
    nimg = n // NCORES
    feats, _ = _run(inputs, trace=False, nimg=nimg)

    if int(np.asarray(inputs.get("is_local", 1))) == 0:
        return feats.astype(np.float32)

    batch_size = int(np.asarray(inputs["batch_size"]))
    seg = np.asarray(inputs["f_obj_to_img"]).astype(np.int64)
    nh = n // 2
    fake, real = feats[:nh], feats[nh:]
    counts = np.bincount(seg, minlength=batch_size).astype(np.float32)
    denom = np.maximum(counts, 1.0)[:, None]
    fsum = np.zeros((batch_size, 8192), np.float32)
    rsum = np.zeros((batch_size, 8192), np.float32)
    np.add.at(fsum, seg, fake)
    np.add.at(rsum, seg, real)
    favg = np.where((counts > 0)[:, None], fsum / denom, 0.0)
    ravg = np.where((counts > 0)[:, None], rsum / denom, 0.0)
    return np.concatenate([favg, ravg], axis=0).astype(np.float32)
